# revision 28
# baseline (speedup 1.0000x reference)
"""Trainium2 Bass kernel for the DINO-style CorrelationLoss (v2.6).

Math:
  loss = dino + 5.0 * corr
  dino = (1/18) * sum_{(t,s) allowed} M[t,s]
  M[t,s] = mean_b LSE(x_s[s,b]/Ts) - mean_b dot(t_p[t,b], x_s[s,b])/Ts
  t_p = softmax((teacher-center)/Tt);  dot(t_p,x) = dots/Z with
  dots = sum_d e_t*x, Z = sum_d e_t, e_t ~ exp((te-c)/Tt - rowmax).

Layout: d-on-partitions. Host transposes each core's shard to
  xbuf[p, q*161 + s*16+b] = x[s, b, q*128+p]   (col 160 of each chunk = 1.0)
  et8 [p, q*32  + t*16+b] = e_t[t, b, q*128+p] (fp8, row-max normalized to
                            192; the scale cancels in dots/Z)
so the d-contraction is the PE's native partition contraction: per 128-d
chunk q, matmul(stationary e_t_q [128,32], moving x_q [128,161])
accumulates dots[(t,b),(s,b)] (+ Z via the ones column) into PSUM over
all 512 chunks -- no DVE elementwise products at all (the baseline
burned ~95us of DVE on them). ACT's only job is the student exp(10x)
stream; ones-stationary matmuls column-sum it for the LSE. PE
col-groups: dots on strips 0/1 (even/odd chunks), LSE on strips 2/3, so
four matmul streams overlap inside the 128x128 array.

Scheduling: with all 8 cores streaming, per-core HBM sustains only
~310 GB/s, so the kernel is DMA-rate/ACT co-bound. Each dma_start's
~2us completion receipt blocks the next DMA on the same DGE ring, so x
segments alternate between the sync-HWDGE and gpsimd-SWDGE rings;
teacher e_t travels as fp8 (half the bytes) on the gpsimd ring between
x segments, cast fp8->bf16 in-flight by the SWDGE (cast-on-DMA), and
the dots matmuls trail two segments so e_t never gates the x stream.

Host does input marshalling (bf16/fp8 casts, center fold, teacher exp
at 1/10 of the bytes, layout transposes) and the final tiny algebra:
psum row combine, log/ratio/means, and the 10x10 crop-0 correlation
block (which needs raw f32 crop-0 rows), as the baseline did.

Per-core traffic: 21.1 MB student + 2.1 MB e_t = 23.2 MB vs the
baseline's 33.6 MB, with DVE/PE/ACT loads of ~1/60/73 us vs the
baseline's ~100/93/99 us.
"""

import numpy as np
import ml_dtypes

import concourse.bass as bass
import concourse.bacc as bacc
import concourse.tile as tile
from concourse import mybir
from concourse.bass_utils import run_bass_kernel_spmd

# problem constants (hardcoded; kernel.py must be self-contained)
NS, NT, B, D = 10, 2, 128, 65536
NCORES = 8
BL = B // NCORES            # 16 samples per core
P = 128                     # d-elements per chunk = partitions
CH = D // P                 # 512 chunks
SCOL = NS * BL              # 160 student (s,b) columns per chunk
XW = SCOL + 1               # +1 ones column (gives Z during the dots matmul)
TCOL = NT * BL              # 32 teacher (t,b) columns per chunk
# chunks per DMA segment: small head (ACT spin-up despite the ~2us
# per-small-DMA cost) and tail (short serial drain), big middle
SEGS = [8, 8, 16, 32, 64, 64, 64, 64, 64, 64, 32, 16, 8, 8]
SEGCAP = max(SEGS)
# teacher e_t pieces (chunk ranges) fetched on the gpsimd SWDGE ring after
# the given x slot; sized so each lands+casts before its dots matmuls
ET_PIECES = [(0, 64), (64, 256), (256, 384), (384, 512)]
ET_AFTER_SLOT = {0: 0, 2: 1, 5: 2, 7: 3}
NWARM = 16
STUDENT_TEMP = 0.1
TEACHER_TEMP = 0.04
MARGIN = 0.7
CORR_WEIGHT = 5.0
ET_CEIL = 192.0             # e_t row max after host normalization

F32 = mybir.dt.float32
BF16 = mybir.dt.bfloat16
FP8 = mybir.dt.float8e4

_CACHED = None


def _build_module():
    nc = bacc.Bacc("TRN2", target_bir_lowering=False, debug=False)
    xbuf = nc.declare_dram_parameter("xbuf", [P, CH * XW], BF16, isOutput=False)
    etbuf = nc.declare_dram_parameter("et", [P, CH * TCOL], FP8, isOutput=False)
    # rows 0-31 / 32-63: dots+Z (even/odd chunks); row 64 / 96: LSE sums
    out = nc.declare_dram_parameter("out", [P, XW], F32, isOutput=True)

    from contextlib import ExitStack

    with tile.TileContext(nc) as tc:
        with ExitStack() as stack:
            consts = stack.enter_context(tc.tile_pool(name="consts", bufs=1))
            et_pool = stack.enter_context(tc.tile_pool(name="etp", bufs=1))
            xseg_pool = stack.enter_context(tc.tile_pool(name="xseg", bufs=5))
            expx_pool = stack.enter_context(tc.tile_pool(name="expx", bufs=2))
            evict_pool = stack.enter_context(tc.tile_pool(name="evict", bufs=1))
            psum_pool = stack.enter_context(
                tc.tile_pool(name="psum", bufs=1, space=bass.MemorySpace.PSUM)
            )

            ones = consts.tile([P, 1], BF16, tag="ones")
            nc.gpsimd.memset(ones[:], 1.0)
            bias0 = consts.tile([P, 1], F32, tag="bias0")
            nc.gpsimd.memset(bias0[:], 0.0)
            junk = consts.tile([P, 512], BF16, tag="junk")
            nc.gpsimd.memset(junk[:], 0.0)
            junks = consts.tile([P, 16], BF16, tag="junks")

            # psum banks: 2 dots accumulators, 2 LSE accumulators, warm-up
            pd = [psum_pool.tile([P, 512], F32, tag=f"pd{g}", name=f"pd{g}")
                  for g in range(2)]
            pl = [psum_pool.tile([P, 512], F32, tag=f"pl{g}", name=f"pl{g}")
                  for g in range(2)]
            pj = psum_pool.tile([P, 512], F32, tag="pj", name="pj")

            c0s = np.cumsum([0] + SEGS)[:-1]
            et8 = et_pool.tile([P, CH * TCOL], FP8, tag="et8")
            et = et_pool.tile([P, CH * TCOL], BF16, tag="et")
            xsegs = []

            def emit_xdma(k):
                xb = xseg_pool.tile([P, SEGCAP, XW], BF16, name="xb")
                a = c0s[k] * XW
                # x rides sync-HWDGE exclusively (the fast ring); e_t rides
                # gpsimd-SWDGE so it never serializes the x receipt chain,
                # with the idle DVE doing the fp8->bf16 cast
                nc.sync.dma_start(xb[:, 0:SEGS[k], :], xbuf[:, a:a + SEGS[k] * XW])
                xsegs.append(xb)
                if k in ET_AFTER_SLOT:
                    ca, cb = ET_PIECES[ET_AFTER_SLOT[k]]
                    a, b = ca * TCOL, cb * TCOL
                    nc.gpsimd.dma_start(et8[:, a:b], etbuf[:, a:b])
                    nc.vector.tensor_copy(et[:, a:b], et8[:, a:b])

            emit_xdma(0)
            nc.scalar.activation(      # dummy: ACT table load under the DMA
                junks[:], junk[:, 0:16], mybir.ActivationFunctionType.Exp,
                bias=bias0[:], scale=1.0,
            )
            emit_xdma(1)
            emit_xdma(2)
            emit_xdma(3)
            emit_xdma(4)
            # PE warm-up so HAM reaches 8/8 before real work lands
            for _ in range(NWARM):
                nc.tensor.matmul(
                    pj[0:32, :], junk[:, 0:32], junk[:],
                    start=True, stop=True, skip_group_check=True,
                    tile_position=(0, 0),
                )

            expxs = []

            def emit_dots(k):
                c0, n = c0s[k], SEGS[k]
                xb = xsegs[k]
                for q in range(n):
                    c = c0 + q
                    g = c & 1
                    nc.tensor.matmul(
                        pd[g][32 * g:32 * g + 32, 0:XW],
                        et[:, c * TCOL:(c + 1) * TCOL],
                        xb[:, q, :],
                        start=(c < 2), stop=(c >= CH - 2),
                        skip_group_check=True,
                        tile_position=(0, 32 * g),
                    )

            def emit_lse(k):
                c0, n = c0s[k], SEGS[k]
                ex = expxs[k]
                for q in range(n):
                    c = c0 + q
                    g = c & 1
                    nc.tensor.matmul(
                        pl[g][64 + 32 * g:65 + 32 * g, 0:XW],
                        ones[:],
                        ex[:, q, :],
                        start=(c < 2), stop=(c >= CH - 2),
                        skip_group_check=True,
                        tile_position=(0, 64 + 32 * g),
                    )

            nseg = len(SEGS)
            for k in range(nseg):
                n = SEGS[k]
                if k + 5 < nseg:
                    emit_xdma(k + 5)
                ex = expx_pool.tile([P, SEGCAP, XW], BF16, name="expx")
                nc.scalar.activation(
                    ex[:, 0:n, :], xsegs[k][:, 0:n, :],
                    mybir.ActivationFunctionType.Exp,
                    bias=bias0[:], scale=1.0 / STUDENT_TEMP,
                )
                expxs.append(ex)
                if k > 0:
                    emit_lse(k - 1)
                if k >= 2:
                    emit_dots(k - 2)
            emit_dots(nseg - 2)
            emit_dots(nseg - 1)
            emit_lse(nseg - 1)

            ev = evict_pool.tile([P, XW], F32, tag="ev")
            nc.vector.tensor_copy(ev[0:32, :], pd[0][0:32, 0:XW])
            nc.scalar.copy(ev[32:64, :], pd[1][32:64, 0:XW])
            nc.vector.tensor_copy(ev[64:65, :], pl[0][64:65, 0:XW])
            nc.scalar.copy(ev[96:97, :], pl[1][96:97, 0:XW])
            nc.sync.dma_start(out[:], ev[:])

    nc.compile()
    return nc


def _get_module():
    global _CACHED
    if _CACHED is None:
        _CACHED = _build_module()
    return _CACHED


def _f32_to_bf16_bits(a):
    """f32 ndarray -> uint16 bf16 bits, round-to-nearest-even."""
    u = np.ascontiguousarray(a, dtype=np.float32).view(np.uint32)
    return ((u + 0x7FFF + ((u >> 16) & 1)) >> 16).astype(np.uint16)


def _prepare_inmaps(student_output, teacher_output, center):
    st_bits = _f32_to_bf16_bits(student_output)          # [NS,B,D]
    # e_t normalized per (t,b) row to max ET_CEIL for fp8 (the scale
    # cancels in dots/Z, so the host never needs to undo it)
    a = (np.asarray(teacher_output, np.float32)
         - np.asarray(center, np.float32)[None]) / TEACHER_TEMP
    a -= a.max(axis=-1, keepdims=True)
    et8 = np.exp(a + np.float32(np.log(ET_CEIL))).astype(ml_dtypes.float8_e4m3)
    et_bits = et8.view(np.uint8)                         # [NT,B,D]
    in_maps = []
    for core in range(NCORES):
        b0 = core * BL
        xb = np.empty((P, CH, XW), np.uint16)
        # [NS,BL,CH,P] -> [P,CH,NS,BL]
        xs = st_bits[:, b0:b0 + BL].reshape(NS, BL, CH, P).transpose(3, 2, 0, 1)
        xb[:, :, :SCOL] = xs.reshape(P, CH, SCOL)
        xb[:, :, SCOL] = 0x3F80                          # bf16 1.0
        ts = et_bits[:, b0:b0 + BL].reshape(NT, BL, CH, P).transpose(3, 2, 0, 1)
        in_maps.append({
            "xbuf": xb.reshape(P, CH * XW).view(ml_dtypes.bfloat16),
            "et": np.ascontiguousarray(ts.reshape(P, CH * TCOL)).view(ml_dtypes.float8_e4m3),
        })
    return in_maps


def kernel(student_output, teacher_output, center):
    nc = _get_module()
    in_maps = _prepare_inmaps(student_output, teacher_output, center)
    res = run_bass_kernel_spmd(nc, in_maps, list(range(NCORES))).results

    dots = np.zeros((NT, NS, B))
    z = np.zeros((NT, B))
    lse_sum = np.zeros((NS, B))
    for core in range(NCORES):
        b0 = core * BL
        o = np.asarray(res[core]["out"], dtype=np.float64)  # [128, 161]
        pair = o[0:32].reshape(NT, BL, XW) + o[32:64].reshape(NT, BL, XW)
        dots[:, :, b0:b0 + BL] = np.einsum(
            "tbsb->tsb", pair[:, :, :SCOL].reshape(NT, BL, NS, BL)
        )
        z[:, b0:b0 + BL] = pair[:, :, SCOL]
        lrow = o[64] + o[96]
        lse_sum[:, b0:b0 + BL] = lrow[:SCOL].reshape(NS, BL)

    lse = np.log(lse_sum)
    term = dots / (z[:, None, :] * STUDENT_TEMP)
    M = -(term.mean(axis=-1) - lse.mean(axis=-1)[None, :])
    skip = np.arange(NT)[:, None] == np.arange(NS)[None, :]
    dino = np.where(skip, 0.0, M).sum() / (NT * NS - min(NT, NS))

    e0 = np.asarray(student_output, dtype=np.float32)[0, :NS].astype(np.float64)
    e0 = e0 / np.maximum(np.linalg.norm(e0, axis=-1, keepdims=True), 1e-12)
    sim = e0 @ e0.T
    iu = np.triu(np.ones((NS, NS)), k=1)
    corr = (np.maximum(sim - (1.0 - MARGIN), 0.0) * iu).sum() / (NS * (NS - 1) // 2)

    return np.float32(dino + CORR_WEIGHT * corr)


# revision 33
# speedup vs baseline: 1.1201x; 1.1201x over previous
"""Trainium2 Bass kernel for the DINO-style CorrelationLoss (v2.6).

Math:
  loss = dino + 5.0 * corr
  dino = (1/18) * sum_{(t,s) allowed} M[t,s]
  M[t,s] = mean_b LSE(x_s[s,b]/Ts) - mean_b dot(t_p[t,b], x_s[s,b])/Ts
  t_p = softmax((teacher-center)/Tt);  dot(t_p,x) = dots/Z with
  dots = sum_d e_t*x, Z = sum_d e_t, e_t ~ exp((te-c)/Tt - rowmax).

Layout: d-on-partitions. Host transposes each core's shard to
  xbuf[p, q*161 + s*16+b] = x[s, b, q*128+p]   (col 160 of each chunk = 1.0)
  et8 [p, q*32  + t*16+b] = e_t[t, b, q*128+p] (fp8, row-max normalized to
                            192; the scale cancels in dots/Z)
so the d-contraction is the PE's native partition contraction: per 128-d
chunk q, matmul(stationary e_t_q [128,32], moving x_q [128,161])
accumulates dots[(t,b),(s,b)] (+ Z via the ones column) into PSUM over
all 512 chunks -- no DVE elementwise products at all (the baseline
burned ~95us of DVE on them). ACT's only job is the student exp(10x)
stream; ones-stationary matmuls column-sum it for the LSE. PE
col-groups: dots on strips 0/1 (even/odd chunks), LSE on strips 2/3, so
four matmul streams overlap inside the 128x128 array.

Scheduling: with all 8 cores streaming, per-core HBM sustains only
~310 GB/s, so the kernel is DMA-rate/ACT co-bound. Each dma_start's
~2us completion receipt blocks the next DMA on the same DGE ring, so x
segments alternate between the sync-HWDGE and gpsimd-SWDGE rings;
teacher e_t travels as fp8 (half the bytes) on the gpsimd ring between
x segments, cast fp8->bf16 in-flight by the SWDGE (cast-on-DMA), and
the dots matmuls trail two segments so e_t never gates the x stream.

Host does input marshalling (bf16/fp8 casts, center fold, teacher exp
at 1/10 of the bytes, layout transposes) and the final tiny algebra:
psum row combine, log/ratio/means, and the 10x10 crop-0 correlation
block (which needs raw f32 crop-0 rows), as the baseline did.

Per-core traffic: 21.1 MB student + 2.1 MB e_t = 23.2 MB vs the
baseline's 33.6 MB, with DVE/PE/ACT loads of ~1/60/73 us vs the
baseline's ~100/93/99 us.
"""

import numpy as np
import ml_dtypes

import concourse.bass as bass
import concourse.bacc as bacc
import concourse.tile as tile
from concourse import mybir
from concourse.bass_utils import run_bass_kernel_spmd

# problem constants (hardcoded; kernel.py must be self-contained)
NS, NT, B, D = 10, 2, 128, 65536
NCORES = 8
BL = B // NCORES            # 16 samples per core
P = 128                     # d-elements per chunk = partitions
CH = D // P                 # 512 chunks
SCOL = NS * BL              # 160 student (s,b) columns per chunk
XW = SCOL + 1               # +1 ones column (gives Z during the dots matmul)
TCOL = NT * BL              # 32 teacher (t,b) columns per chunk
# chunks per DMA segment: small head (ACT spin-up despite the ~2us
# per-small-DMA cost) and tail (short serial drain), big middle
SEGS = [8, 8, 16, 32, 64, 64, 64, 64, 64, 64, 32, 16, 16]
SEGCAP = max(SEGS)
# teacher e_t pieces (chunk ranges) fetched on the gpsimd SWDGE ring after
# the given x slot; sized so each lands+casts before its dots matmuls
ET_PIECES = [(0, 64), (64, 256), (256, 384), (384, 512)]
ET_AFTER_SLOT = {0: 0, 2: 1, 5: 2, 7: 3}
NWARM = 16
STUDENT_TEMP = 0.1
TEACHER_TEMP = 0.04
MARGIN = 0.7
CORR_WEIGHT = 5.0
ET_CEIL = 192.0             # e_t row max after host normalization

F32 = mybir.dt.float32
BF16 = mybir.dt.bfloat16
FP8 = mybir.dt.float8e4

_CACHED = None


def _build_module():
    nc = bacc.Bacc("TRN2", target_bir_lowering=False, debug=False)
    xbuf = nc.declare_dram_parameter("xbuf", [P, CH * XW], BF16, isOutput=False)
    etbuf = nc.declare_dram_parameter("et", [P, CH * TCOL], FP8, isOutput=False)
    # rows 0-31 / 32-63: dots+Z (even/odd chunks); row 64 / 96: LSE sums
    out = nc.declare_dram_parameter("out", [P, XW], F32, isOutput=True)

    from contextlib import ExitStack

    with tile.TileContext(nc) as tc:
        with ExitStack() as stack:
            consts = stack.enter_context(tc.tile_pool(name="consts", bufs=1))
            et_pool = stack.enter_context(tc.tile_pool(name="etp", bufs=1))
            xseg_pool = stack.enter_context(tc.tile_pool(name="xseg", bufs=4))
            expx_pool = stack.enter_context(tc.tile_pool(name="expx", bufs=2))
            evict_pool = stack.enter_context(tc.tile_pool(name="evict", bufs=1))
            psum_pool = stack.enter_context(
                tc.tile_pool(name="psum", bufs=1, space=bass.MemorySpace.PSUM)
            )

            ones = consts.tile([P, 1], BF16, tag="ones")
            nc.gpsimd.memset(ones[:], 1.0)
            bias0 = consts.tile([P, 1], F32, tag="bias0")
            nc.gpsimd.memset(bias0[:], 0.0)
            junk = consts.tile([P, 512], BF16, tag="junk")
            nc.gpsimd.memset(junk[:], 0.0)
            junks = consts.tile([P, 16], BF16, tag="junks")

            # psum banks: 2 dots accumulators, 2 LSE accumulators, warm-up
            pd = [psum_pool.tile([P, 512], F32, tag=f"pd{g}", name=f"pd{g}")
                  for g in range(2)]
            pl = [psum_pool.tile([P, 512], F32, tag=f"pl{g}", name=f"pl{g}")
                  for g in range(2)]
            pj = psum_pool.tile([P, 512], F32, tag="pj", name="pj")

            c0s = np.cumsum([0] + SEGS)[:-1]
            et8 = et_pool.tile([P, CH * TCOL], FP8, tag="et8")
            et = et_pool.tile([P, CH * TCOL], BF16, tag="et")
            xsegs = []

            def emit_xdma(k):
                xb = xseg_pool.tile([P, SEGCAP, XW], BF16, name="xb")
                a = c0s[k] * XW
                # x rides sync-HWDGE exclusively (the fast ring); e_t rides
                # gpsimd-SWDGE so it never serializes the x receipt chain,
                # with the idle DVE doing the fp8->bf16 cast
                nc.sync.dma_start(xb[:, 0:SEGS[k], :], xbuf[:, a:a + SEGS[k] * XW])
                xsegs.append(xb)

            emit_xdma(0)
            nc.scalar.activation(      # dummy: ACT table load under the DMA
                junks[:], junk[:, 0:16], mybir.ActivationFunctionType.Exp,
                bias=bias0[:], scale=1.0,
            )
            emit_xdma(1)
            emit_xdma(2)
            emit_xdma(3)
            # PE warm-up so HAM reaches 8/8 before real work lands
            for _ in range(NWARM):
                nc.tensor.matmul(
                    pj[0:32, :], junk[:, 0:32], junk[:],
                    start=True, stop=True, skip_group_check=True,
                    tile_position=(0, 0),
                )

            expxs = []

            def emit_dots(k):
                c0, n = c0s[k], SEGS[k]
                xb = xsegs[k]
                for q in range(n):
                    c = c0 + q
                    g = c & 1
                    nc.tensor.matmul(
                        pd[g][32 * g:32 * g + 32, 0:XW],
                        et[:, c * TCOL:(c + 1) * TCOL],
                        xb[:, q, :],
                        start=(c < 2), stop=(c >= CH - 2),
                        skip_group_check=True,
                        tile_position=(0, 32 * g),
                    )

            def emit_lse(k):
                c0, n = c0s[k], SEGS[k]
                ex = expxs[k]
                for q in range(n):
                    c = c0 + q
                    g = c & 1
                    nc.tensor.matmul(
                        pl[g][64 + 32 * g:65 + 32 * g, 0:XW],
                        ones[:],
                        ex[:, q, :],
                        start=(c < 2), stop=(c >= CH - 2),
                        skip_group_check=True,
                        tile_position=(0, 64 + 32 * g),
                    )

            nseg = len(SEGS)
            for k in range(nseg):
                n = SEGS[k]
                if k + 4 < nseg:
                    emit_xdma(k + 4)
                if k in ET_AFTER_SLOT:
                    ca, cb = ET_PIECES[ET_AFTER_SLOT[k]]
                    a, b = ca * TCOL, cb * TCOL
                    nc.gpsimd.dma_start(et8[:, a:b], etbuf[:, a:b])
                    nc.vector.tensor_copy(et[:, a:b], et8[:, a:b])
                ex = expx_pool.tile([P, SEGCAP, XW], BF16, name="expx")
                nc.scalar.activation(
                    ex[:, 0:n, :], xsegs[k][:, 0:n, :],
                    mybir.ActivationFunctionType.Exp,
                    bias=bias0[:], scale=1.0 / STUDENT_TEMP,
                )
                expxs.append(ex)
                if k > 0:
                    emit_lse(k - 1)
                if k >= 2:
                    emit_dots(k - 2)
            emit_dots(nseg - 2)
            emit_dots(nseg - 1)
            emit_lse(nseg - 1)

            ev = evict_pool.tile([P, XW], F32, tag="ev")
            nc.vector.tensor_copy(ev[0:32, :], pd[0][0:32, 0:XW])
            nc.vector.tensor_copy(ev[32:64, :], pd[1][32:64, 0:XW])
            nc.vector.tensor_copy(ev[64:65, :], pl[0][64:65, 0:XW])
            nc.vector.tensor_copy(ev[96:97, :], pl[1][96:97, 0:XW])
            nc.sync.dma_start(out[:], ev[:])

    nc.compile()
    return nc


def _get_module():
    global _CACHED
    if _CACHED is None:
        _CACHED = _build_module()
    return _CACHED


def _f32_to_bf16_bits(a):
    """f32 ndarray -> uint16 bf16 bits, round-to-nearest-even."""
    u = np.ascontiguousarray(a, dtype=np.float32).view(np.uint32)
    return ((u + 0x7FFF + ((u >> 16) & 1)) >> 16).astype(np.uint16)


def _prepare_inmaps(student_output, teacher_output, center):
    st_bits = _f32_to_bf16_bits(student_output)          # [NS,B,D]
    # e_t normalized per (t,b) row to max ET_CEIL for fp8 (the scale
    # cancels in dots/Z, so the host never needs to undo it)
    a = (np.asarray(teacher_output, np.float32)
         - np.asarray(center, np.float32)[None]) / TEACHER_TEMP
    a -= a.max(axis=-1, keepdims=True)
    et8 = np.exp(a + np.float32(np.log(ET_CEIL))).astype(ml_dtypes.float8_e4m3)
    et_bits = et8.view(np.uint8)                         # [NT,B,D]
    in_maps = []
    for core in range(NCORES):
        b0 = core * BL
        xb = np.empty((P, CH, XW), np.uint16)
        # [NS,BL,CH,P] -> [P,CH,NS,BL]
        xs = st_bits[:, b0:b0 + BL].reshape(NS, BL, CH, P).transpose(3, 2, 0, 1)
        xb[:, :, :SCOL] = xs.reshape(P, CH, SCOL)
        xb[:, :, SCOL] = 0x3F80                          # bf16 1.0
        ts = et_bits[:, b0:b0 + BL].reshape(NT, BL, CH, P).transpose(3, 2, 0, 1)
        in_maps.append({
            "xbuf": xb.reshape(P, CH * XW).view(ml_dtypes.bfloat16),
            "et": np.ascontiguousarray(ts.reshape(P, CH * TCOL)).view(ml_dtypes.float8_e4m3),
        })
    return in_maps


def kernel(student_output, teacher_output, center):
    nc = _get_module()
    in_maps = _prepare_inmaps(student_output, teacher_output, center)
    res = run_bass_kernel_spmd(nc, in_maps, list(range(NCORES))).results

    dots = np.zeros((NT, NS, B))
    z = np.zeros((NT, B))
    lse_sum = np.zeros((NS, B))
    for core in range(NCORES):
        b0 = core * BL
        o = np.asarray(res[core]["out"], dtype=np.float64)  # [128, 161]
        pair = o[0:32].reshape(NT, BL, XW) + o[32:64].reshape(NT, BL, XW)
        dots[:, :, b0:b0 + BL] = np.einsum(
            "tbsb->tsb", pair[:, :, :SCOL].reshape(NT, BL, NS, BL)
        )
        z[:, b0:b0 + BL] = pair[:, :, SCOL]
        lrow = o[64] + o[96]
        lse_sum[:, b0:b0 + BL] = lrow[:SCOL].reshape(NS, BL)

    lse = np.log(lse_sum)
    term = dots / (z[:, None, :] * STUDENT_TEMP)
    M = -(term.mean(axis=-1) - lse.mean(axis=-1)[None, :])
    skip = np.arange(NT)[:, None] == np.arange(NS)[None, :]
    dino = np.where(skip, 0.0, M).sum() / (NT * NS - min(NT, NS))

    e0 = np.asarray(student_output, dtype=np.float32)[0, :NS].astype(np.float64)
    e0 = e0 / np.maximum(np.linalg.norm(e0, axis=-1, keepdims=True), 1e-12)
    sim = e0 @ e0.T
    iu = np.triu(np.ones((NS, NS)), k=1)
    corr = (np.maximum(sim - (1.0 - MARGIN), 0.0) * iu).sum() / (NS * (NS - 1) // 2)

    return np.float32(dino + CORR_WEIGHT * corr)


# revision 37
# speedup vs baseline: 1.1325x; 1.0111x over previous
"""Trainium2 Bass kernel for the DINO-style CorrelationLoss.

Math:
  loss = dino + 5.0 * corr
  dino = (1/18) * sum_{(t,s) allowed} M[t,s]
  M[t,s] = mean_b LSE(x_s[s,b]/Ts) - mean_b dot(t_p[t,b], x_s[s,b])/Ts
  t_p = softmax((teacher-center)/Tt);  dot(t_p,x) = dots/Z with
  dots = sum_d e_t*x, Z = sum_d e_t, e_t ~ exp((te-c)/Tt - rowmax).

Layout: d-on-partitions. Host transposes each core's shard to
  xbuf[p, q*161 + s*16+b] = x[s, b, q*128+p]   (col 160 of each chunk = 1.0)
  et8 [p, q*32  + t*16+b] = e_t[t, b, q*128+p] (fp8, row-max normalized to
                            192; the scale cancels in dots/Z)
so the d-contraction is the PE's native partition contraction: per 128-d
chunk q, matmul(stationary e_t_q [128,32], moving x_q [128,161])
accumulates dots[(t,b),(s,b)] (+ Z via the ones column) into PSUM over
all 512 chunks -- no DVE elementwise products at all (the baseline
burned ~95us of DVE on them). ACT's only job is the student exp(10x)
stream; ones-stationary matmuls column-sum it for the LSE. PE
col-groups: dots on strips 0/1 (even/odd chunks), LSE on strips 2/3, so
four matmul streams overlap inside the 128x128 array.

Scheduling: with all 8 cores streaming, per-core HBM sustains only
~300-310 GB/s, so the kernel is DMA-rate/ACT co-bound (ACT consumes x
at ~307 GB/s). The x stream rides the sync-HWDGE ring exclusively,
ramping segment sizes 8->64 chunks (small DMAs pay a ~2-3us fixed cost
each, but let ACT start early); teacher e_t travels as fp8 (half the
bytes) on the gpsimd-SWDGE ring so it never serializes the x receipt
chain, the idle DVE casts it fp8->bf16, and the dots matmuls trail two
segments so e_t never gates the x stream.

Host does input marshalling (bf16/fp8 casts, center fold, teacher exp
at 1/10 of the bytes, layout transposes) and the final tiny algebra:
psum row combine, log/ratio/means, and the 10x10 crop-0 correlation
block (which needs raw f32 crop-0 rows), as the baseline did.

Per-core traffic: 21.1 MB student + 2.1 MB e_t = 23.2 MB vs the
baseline's 33.6 MB, with DVE/PE/ACT loads of ~1/60/73 us vs the
baseline's ~100/93/99 us.
"""

import numpy as np
import ml_dtypes

import concourse.bass as bass
import concourse.bacc as bacc
import concourse.tile as tile
from concourse import mybir
from concourse.bass_utils import run_bass_kernel_spmd

# problem constants (hardcoded; kernel.py must be self-contained)
NS, NT, B, D = 10, 2, 128, 65536
NCORES = 8
BL = B // NCORES            # 16 samples per core
P = 128                     # d-elements per chunk = partitions
CH = D // P                 # 512 chunks
SCOL = NS * BL              # 160 student (s,b) columns per chunk
XW = SCOL + 1               # +1 ones column (gives Z during the dots matmul)
TCOL = NT * BL              # 32 teacher (t,b) columns per chunk
# chunks per DMA segment: small head (ACT spin-up despite the ~2us
# per-small-DMA cost) and tail (short serial drain), big middle
SEGS = [8, 8, 16, 32, 64, 64, 64, 64, 64, 64, 32, 16, 8, 8]
SEGCAP = max(SEGS)
# teacher e_t pieces (chunk ranges) fetched on the gpsimd SWDGE ring after
# the given x slot: as late as their dots deadlines allow (dots trail two
# slots), so the fetches stay clear of the bandwidth-starved ramp
ET_PIECES = [(0, 64), (64, 256), (256, 384), (384, 512)]
ET_AFTER_SLOT = {0: 0, 4: 1, 6: 2, 8: 3}
NWARM = 16
STUDENT_TEMP = 0.1
TEACHER_TEMP = 0.04
MARGIN = 0.7
CORR_WEIGHT = 5.0
ET_CEIL = 192.0             # e_t row max after host normalization

F32 = mybir.dt.float32
BF16 = mybir.dt.bfloat16
FP8 = mybir.dt.float8e4

_CACHED = None


def _build_module():
    nc = bacc.Bacc("TRN2", target_bir_lowering=False, debug=False)
    xbuf = nc.declare_dram_parameter("xbuf", [P, CH * XW], BF16, isOutput=False)
    etbuf = nc.declare_dram_parameter("et", [P, CH * TCOL], FP8, isOutput=False)
    # rows 0-31 / 32-63: dots+Z (even/odd chunks); row 64 / 96: LSE sums
    out = nc.declare_dram_parameter("out", [P, XW], F32, isOutput=True)

    from contextlib import ExitStack

    with tile.TileContext(nc) as tc:
        with ExitStack() as stack:
            consts = stack.enter_context(tc.tile_pool(name="consts", bufs=1))
            et_pool = stack.enter_context(tc.tile_pool(name="etp", bufs=1))
            xseg_pool = stack.enter_context(tc.tile_pool(name="xseg", bufs=4))
            expx_pool = stack.enter_context(tc.tile_pool(name="expx", bufs=2))
            evict_pool = stack.enter_context(tc.tile_pool(name="evict", bufs=1))
            psum_pool = stack.enter_context(
                tc.tile_pool(name="psum", bufs=1, space=bass.MemorySpace.PSUM)
            )

            ones = consts.tile([P, 1], BF16, tag="ones")
            nc.gpsimd.memset(ones[:], 1.0)
            bias0 = consts.tile([P, 1], F32, tag="bias0")
            nc.gpsimd.memset(bias0[:], 0.0)
            junk = consts.tile([P, 512], BF16, tag="junk")
            nc.gpsimd.memset(junk[:], 0.0)
            junks = consts.tile([P, 16], BF16, tag="junks")

            # psum banks: 2 dots accumulators, 2 LSE accumulators, warm-up
            pd = [psum_pool.tile([P, 512], F32, tag=f"pd{g}", name=f"pd{g}")
                  for g in range(2)]
            pl = [psum_pool.tile([P, 512], F32, tag=f"pl{g}", name=f"pl{g}")
                  for g in range(2)]
            pj = psum_pool.tile([P, 512], F32, tag="pj", name="pj")

            c0s = np.cumsum([0] + SEGS)[:-1]
            et8 = et_pool.tile([P, CH * TCOL], FP8, tag="et8")
            et = et_pool.tile([P, CH * TCOL], BF16, tag="et")
            xsegs = []

            def emit_xdma(k):
                xb = xseg_pool.tile([P, SEGCAP, XW], BF16, name="xb")
                a = c0s[k] * XW
                # x rides sync-HWDGE exclusively (the fast ring); e_t rides
                # gpsimd-SWDGE so it never serializes the x receipt chain,
                # with the idle DVE doing the fp8->bf16 cast
                nc.sync.dma_start(xb[:, 0:SEGS[k], :], xbuf[:, a:a + SEGS[k] * XW])
                xsegs.append(xb)

            emit_xdma(0)
            nc.scalar.activation(      # dummy: ACT table load under the DMA
                junks[:], junk[:, 0:16], mybir.ActivationFunctionType.Exp,
                bias=bias0[:], scale=1.0,
            )
            emit_xdma(1)
            emit_xdma(2)
            emit_xdma(3)
            # PE warm-up so HAM reaches 8/8 before real work lands
            for _ in range(NWARM):
                nc.tensor.matmul(
                    pj[0:32, :], junk[:, 0:32], junk[:],
                    start=True, stop=True, skip_group_check=True,
                    tile_position=(0, 0),
                )

            expxs = []

            def emit_dots(k):
                c0, n = c0s[k], SEGS[k]
                xb = xsegs[k]
                for q in range(n):
                    c = c0 + q
                    g = c & 1
                    nc.tensor.matmul(
                        pd[g][32 * g:32 * g + 32, 0:XW],
                        et[:, c * TCOL:(c + 1) * TCOL],
                        xb[:, q, :],
                        start=(c < 2), stop=(c >= CH - 2),
                        skip_group_check=True,
                        tile_position=(0, 32 * g),
                    )

            def emit_lse(k):
                c0, n = c0s[k], SEGS[k]
                ex = expxs[k]
                for q in range(n):
                    c = c0 + q
                    g = c & 1
                    nc.tensor.matmul(
                        pl[g][64 + 32 * g:65 + 32 * g, 0:XW],
                        ones[:],
                        ex[:, q, :],
                        start=(c < 2), stop=(c >= CH - 2),
                        skip_group_check=True,
                        tile_position=(0, 64 + 32 * g),
                    )

            nseg = len(SEGS)
            for k in range(nseg):
                n = SEGS[k]
                if k + 4 < nseg:
                    emit_xdma(k + 4)
                if k in ET_AFTER_SLOT:
                    ca, cb = ET_PIECES[ET_AFTER_SLOT[k]]
                    a, b = ca * TCOL, cb * TCOL
                    nc.gpsimd.dma_start(et8[:, a:b], etbuf[:, a:b])
                    nc.vector.tensor_copy(et[:, a:b], et8[:, a:b])
                ex = expx_pool.tile([P, SEGCAP, XW], BF16, name="expx")
                nc.scalar.activation(
                    ex[:, 0:n, :], xsegs[k][:, 0:n, :],
                    mybir.ActivationFunctionType.Exp,
                    bias=bias0[:], scale=1.0 / STUDENT_TEMP,
                )
                expxs.append(ex)
                if k > 0:
                    emit_lse(k - 1)
                if k >= 2:
                    emit_dots(k - 2)
            emit_dots(nseg - 2)
            emit_dots(nseg - 1)
            emit_lse(nseg - 1)

            # evictions split across the (idle) vector and scalar engines
            ev = evict_pool.tile([P, XW], F32, tag="ev")
            nc.vector.tensor_copy(ev[0:32, :], pd[0][0:32, 0:XW])
            nc.scalar.copy(ev[32:64, :], pd[1][32:64, 0:XW])
            nc.vector.tensor_copy(ev[64:65, :], pl[0][64:65, 0:XW])
            nc.scalar.copy(ev[96:97, :], pl[1][96:97, 0:XW])
            nc.sync.dma_start(out[:], ev[:])

    nc.compile()
    return nc


def _get_module():
    global _CACHED
    if _CACHED is None:
        _CACHED = _build_module()
    return _CACHED


def _f32_to_bf16_bits(a):
    """f32 ndarray -> uint16 bf16 bits, round-to-nearest-even."""
    u = np.ascontiguousarray(a, dtype=np.float32).view(np.uint32)
    return ((u + 0x7FFF + ((u >> 16) & 1)) >> 16).astype(np.uint16)


def _prepare_inmaps(student_output, teacher_output, center):
    st_bits = _f32_to_bf16_bits(student_output)          # [NS,B,D]
    # e_t normalized per (t,b) row to max ET_CEIL for fp8 (the scale
    # cancels in dots/Z, so the host never needs to undo it)
    a = (np.asarray(teacher_output, np.float32)
         - np.asarray(center, np.float32)[None]) / TEACHER_TEMP
    a -= a.max(axis=-1, keepdims=True)
    et8 = np.exp(a + np.float32(np.log(ET_CEIL))).astype(ml_dtypes.float8_e4m3)
    et_bits = et8.view(np.uint8)                         # [NT,B,D]
    in_maps = []
    for core in range(NCORES):
        b0 = core * BL
        xb = np.empty((P, CH, XW), np.uint16)
        # [NS,BL,CH,P] -> [P,CH,NS,BL]
        xs = st_bits[:, b0:b0 + BL].reshape(NS, BL, CH, P).transpose(3, 2, 0, 1)
        xb[:, :, :SCOL] = xs.reshape(P, CH, SCOL)
        xb[:, :, SCOL] = 0x3F80                          # bf16 1.0
        ts = et_bits[:, b0:b0 + BL].reshape(NT, BL, CH, P).transpose(3, 2, 0, 1)
        in_maps.append({
            "xbuf": xb.reshape(P, CH * XW).view(ml_dtypes.bfloat16),
            "et": np.ascontiguousarray(ts.reshape(P, CH * TCOL)).view(ml_dtypes.float8_e4m3),
        })
    return in_maps


def kernel(student_output, teacher_output, center):
    nc = _get_module()
    in_maps = _prepare_inmaps(student_output, teacher_output, center)
    res = run_bass_kernel_spmd(nc, in_maps, list(range(NCORES))).results

    dots = np.zeros((NT, NS, B))
    z = np.zeros((NT, B))
    lse_sum = np.zeros((NS, B))
    for core in range(NCORES):
        b0 = core * BL
        o = np.asarray(res[core]["out"], dtype=np.float64)  # [128, 161]
        pair = o[0:32].reshape(NT, BL, XW) + o[32:64].reshape(NT, BL, XW)
        dots[:, :, b0:b0 + BL] = np.einsum(
            "tbsb->tsb", pair[:, :, :SCOL].reshape(NT, BL, NS, BL)
        )
        z[:, b0:b0 + BL] = pair[:, :, SCOL]
        lrow = o[64] + o[96]
        lse_sum[:, b0:b0 + BL] = lrow[:SCOL].reshape(NS, BL)

    lse = np.log(lse_sum)
    term = dots / (z[:, None, :] * STUDENT_TEMP)
    M = -(term.mean(axis=-1) - lse.mean(axis=-1)[None, :])
    skip = np.arange(NT)[:, None] == np.arange(NS)[None, :]
    dino = np.where(skip, 0.0, M).sum() / (NT * NS - min(NT, NS))

    e0 = np.asarray(student_output, dtype=np.float32)[0, :NS].astype(np.float64)
    e0 = e0 / np.maximum(np.linalg.norm(e0, axis=-1, keepdims=True), 1e-12)
    sim = e0 @ e0.T
    iu = np.triu(np.ones((NS, NS)), k=1)
    corr = (np.maximum(sim - (1.0 - MARGIN), 0.0) * iu).sum() / (NS * (NS - 1) // 2)

    return np.float32(dino + CORR_WEIGHT * corr)


# revision 38
# speedup vs baseline: 1.1708x; 1.0338x over previous
"""Trainium2 Bass kernel for the DINO-style CorrelationLoss.

Math:
  loss = dino + 5.0 * corr
  dino = (1/18) * sum_{(t,s) allowed} M[t,s]
  M[t,s] = mean_b LSE(x_s[s,b]/Ts) - mean_b dot(t_p[t,b], x_s[s,b])/Ts
  t_p = softmax((teacher-center)/Tt);  dot(t_p,x) = dots/Z with
  dots = sum_d e_t*x, Z = sum_d e_t, e_t ~ exp((te-c)/Tt - rowmax).

Layout: d-on-partitions. Host transposes each core's shard to
  xbuf[p, q*161 + s*16+b] = x[s, b, q*128+p]   (col 160 of each chunk = 1.0)
  et8 [p, q*32  + t*16+b] = e_t[t, b, q*128+p] (fp8, row-max normalized to
                            192; the scale cancels in dots/Z)
so the d-contraction is the PE's native partition contraction: per 128-d
chunk q, matmul(stationary e_t_q [128,32], moving x_q [128,161])
accumulates dots[(t,b),(s,b)] (+ Z via the ones column) into PSUM over
all 512 chunks -- no DVE elementwise products at all (the baseline
burned ~95us of DVE on them). ACT's only job is the student exp(10x)
stream; ones-stationary matmuls column-sum it for the LSE. PE
col-groups: dots on strips 0/1 (even/odd chunks), LSE on strips 2/3, so
four matmul streams overlap inside the 128x128 array.

Scheduling: with all 8 cores streaming, per-core HBM sustains only
~300-310 GB/s, so the kernel is DMA-rate/ACT co-bound (ACT consumes x
at ~307 GB/s). The x stream rides the sync-HWDGE ring exclusively,
ramping segment sizes 8->64 chunks (small DMAs pay a ~2-3us fixed cost
each, but let ACT start early); teacher e_t travels as fp8 (half the
bytes) on the gpsimd-SWDGE ring so it never serializes the x receipt
chain, the idle DVE casts it fp8->bf16, and the dots matmuls trail two
segments so e_t never gates the x stream.

Host does input marshalling (bf16/fp8 casts, center fold, teacher exp
at 1/10 of the bytes, layout transposes) and the final tiny algebra:
psum row combine, log/ratio/means, and the 10x10 crop-0 correlation
block (which needs raw f32 crop-0 rows), as the baseline did.

Per-core traffic: 21.1 MB student + 2.1 MB e_t = 23.2 MB vs the
baseline's 33.6 MB, with DVE/PE/ACT loads of ~1/60/73 us vs the
baseline's ~100/93/99 us.
"""

import numpy as np
import ml_dtypes

import concourse.bass as bass
import concourse.bacc as bacc
import concourse.tile as tile
from concourse import mybir
from concourse.bass_utils import run_bass_kernel_spmd

# problem constants (hardcoded; kernel.py must be self-contained)
NS, NT, B, D = 10, 2, 128, 65536
NCORES = 8
BL = B // NCORES            # 16 samples per core
P = 128                     # d-elements per chunk = partitions
CH = D // P                 # 512 chunks
SCOL = NS * BL              # 160 student (s,b) columns per chunk
XW = SCOL + 1               # +1 ones column (gives Z during the dots matmul)
TCOL = NT * BL              # 32 teacher (t,b) columns per chunk
# chunks per DMA segment: small head (ACT spin-up despite the ~2us
# per-small-DMA cost) and tail (short serial drain), big middle
SEGS = [8, 8, 16, 32, 64, 64, 64, 64, 64, 64, 32, 16, 8, 8]
SEGCAP = max(SEGS)
# teacher e_t pieces (chunk ranges) fetched on the gpsimd SWDGE ring after
# the given x slot: as late as their dots deadlines allow (dots trail two
# slots), so the fetches stay clear of the bandwidth-starved ramp
ET_PIECES = [(0, 64), (64, 256), (256, 384), (384, 512)]
ET_AFTER_SLOT = {0: 0, 4: 1, 6: 2, 8: 3}
NWARM = 16
STUDENT_TEMP = 0.1
TEACHER_TEMP = 0.04
MARGIN = 0.7
CORR_WEIGHT = 5.0
ET_CEIL = 192.0             # e_t row max after host normalization

F32 = mybir.dt.float32
BF16 = mybir.dt.bfloat16
FP8 = mybir.dt.float8e4

_CACHED = None


def _build_module():
    nc = bacc.Bacc("TRN2", target_bir_lowering=False, debug=False)
    xbuf = nc.declare_dram_parameter("xbuf", [P, CH * XW], BF16, isOutput=False)
    etbuf = nc.declare_dram_parameter("et", [P, CH * TCOL], FP8, isOutput=False)
    # rows 0-31 / 32-63: dots+Z (even/odd chunks); row 64 / 96: LSE sums
    out = nc.declare_dram_parameter("out", [P, XW], F32, isOutput=True)

    from contextlib import ExitStack

    with tile.TileContext(nc) as tc:
        with ExitStack() as stack:
            consts = stack.enter_context(tc.tile_pool(name="consts", bufs=1))
            et_pool = stack.enter_context(tc.tile_pool(name="etp", bufs=1))
            xseg_pool = stack.enter_context(tc.tile_pool(name="xseg", bufs=4))
            expx_pool = stack.enter_context(tc.tile_pool(name="expx", bufs=2))
            evict_pool = stack.enter_context(tc.tile_pool(name="evict", bufs=1))
            psum_pool = stack.enter_context(
                tc.tile_pool(name="psum", bufs=1, space=bass.MemorySpace.PSUM)
            )

            ones = consts.tile([P, 1], BF16, tag="ones")
            nc.gpsimd.memset(ones[:], 1.0)
            bias0 = consts.tile([P, 1], F32, tag="bias0")
            nc.gpsimd.memset(bias0[:], 0.0)
            junk = consts.tile([P, 512], BF16, tag="junk")
            nc.gpsimd.memset(junk[:], 0.0)
            junks = consts.tile([P, 16], BF16, tag="junks")

            # psum banks: 2 dots accumulators, 2 LSE accumulators, warm-up
            pd = [psum_pool.tile([P, 512], F32, tag=f"pd{g}", name=f"pd{g}")
                  for g in range(2)]
            pl = [psum_pool.tile([P, 512], F32, tag=f"pl{g}", name=f"pl{g}")
                  for g in range(2)]
            pj = psum_pool.tile([P, 512], F32, tag="pj", name="pj")

            c0s = np.cumsum([0] + SEGS)[:-1]
            et8 = et_pool.tile([P, CH * TCOL], FP8, tag="et8")
            et = et_pool.tile([P, CH * TCOL], BF16, tag="et")
            xsegs = []

            def emit_xdma(k):
                xb = xseg_pool.tile([P, SEGCAP, XW], BF16, name="xb")
                a = c0s[k] * XW
                # the two tiny head segments ride gpsimd-SWDGE so the sync
                # ring starts its big-segment chain immediately; all other x
                # stays on sync-HWDGE (the fast ring); e_t also rides gpsimd
                # so it never serializes the x receipt chain, with the idle
                # DVE doing the fp8->bf16 cast
                ring = nc.gpsimd if k < 2 else nc.sync
                ring.dma_start(xb[:, 0:SEGS[k], :], xbuf[:, a:a + SEGS[k] * XW])
                xsegs.append(xb)

            emit_xdma(0)
            nc.scalar.activation(      # dummy: ACT table load under the DMA
                junks[:], junk[:, 0:16], mybir.ActivationFunctionType.Exp,
                bias=bias0[:], scale=1.0,
            )
            emit_xdma(1)
            emit_xdma(2)
            emit_xdma(3)
            # PE warm-up so HAM reaches 8/8 before real work lands
            for _ in range(NWARM):
                nc.tensor.matmul(
                    pj[0:32, :], junk[:, 0:32], junk[:],
                    start=True, stop=True, skip_group_check=True,
                    tile_position=(0, 0),
                )

            expxs = []

            def emit_dots(k):
                c0, n = c0s[k], SEGS[k]
                xb = xsegs[k]
                for q in range(n):
                    c = c0 + q
                    g = c & 1
                    nc.tensor.matmul(
                        pd[g][32 * g:32 * g + 32, 0:XW],
                        et[:, c * TCOL:(c + 1) * TCOL],
                        xb[:, q, :],
                        start=(c < 2), stop=(c >= CH - 2),
                        skip_group_check=True,
                        tile_position=(0, 32 * g),
                    )

            def emit_lse(k):
                c0, n = c0s[k], SEGS[k]
                ex = expxs[k]
                for q in range(n):
                    c = c0 + q
                    g = c & 1
                    nc.tensor.matmul(
                        pl[g][64 + 32 * g:65 + 32 * g, 0:XW],
                        ones[:],
                        ex[:, q, :],
                        start=(c < 2), stop=(c >= CH - 2),
                        skip_group_check=True,
                        tile_position=(0, 64 + 32 * g),
                    )

            nseg = len(SEGS)
            for k in range(nseg):
                n = SEGS[k]
                if k + 4 < nseg:
                    emit_xdma(k + 4)
                if k in ET_AFTER_SLOT:
                    ca, cb = ET_PIECES[ET_AFTER_SLOT[k]]
                    a, b = ca * TCOL, cb * TCOL
                    nc.gpsimd.dma_start(et8[:, a:b], etbuf[:, a:b])
                    nc.vector.tensor_copy(et[:, a:b], et8[:, a:b])
                ex = expx_pool.tile([P, SEGCAP, XW], BF16, name="expx")
                nc.scalar.activation(
                    ex[:, 0:n, :], xsegs[k][:, 0:n, :],
                    mybir.ActivationFunctionType.Exp,
                    bias=bias0[:], scale=1.0 / STUDENT_TEMP,
                )
                expxs.append(ex)
                if k > 0:
                    emit_lse(k - 1)
                if k >= 2:
                    emit_dots(k - 2)
            emit_dots(nseg - 2)
            emit_dots(nseg - 1)
            emit_lse(nseg - 1)

            # evictions split across the (idle) vector and scalar engines
            ev = evict_pool.tile([P, XW], F32, tag="ev")
            nc.vector.tensor_copy(ev[0:32, :], pd[0][0:32, 0:XW])
            nc.scalar.copy(ev[32:64, :], pd[1][32:64, 0:XW])
            nc.vector.tensor_copy(ev[64:65, :], pl[0][64:65, 0:XW])
            nc.scalar.copy(ev[96:97, :], pl[1][96:97, 0:XW])
            nc.sync.dma_start(out[:], ev[:])

    nc.compile()
    return nc


def _get_module():
    global _CACHED
    if _CACHED is None:
        _CACHED = _build_module()
    return _CACHED


def _f32_to_bf16_bits(a):
    """f32 ndarray -> uint16 bf16 bits, round-to-nearest-even."""
    u = np.ascontiguousarray(a, dtype=np.float32).view(np.uint32)
    return ((u + 0x7FFF + ((u >> 16) & 1)) >> 16).astype(np.uint16)


def _prepare_inmaps(student_output, teacher_output, center):
    st_bits = _f32_to_bf16_bits(student_output)          # [NS,B,D]
    # e_t normalized per (t,b) row to max ET_CEIL for fp8 (the scale
    # cancels in dots/Z, so the host never needs to undo it)
    a = (np.asarray(teacher_output, np.float32)
         - np.asarray(center, np.float32)[None]) / TEACHER_TEMP
    a -= a.max(axis=-1, keepdims=True)
    et8 = np.exp(a + np.float32(np.log(ET_CEIL))).astype(ml_dtypes.float8_e4m3)
    et_bits = et8.view(np.uint8)                         # [NT,B,D]
    in_maps = []
    for core in range(NCORES):
        b0 = core * BL
        xb = np.empty((P, CH, XW), np.uint16)
        # [NS,BL,CH,P] -> [P,CH,NS,BL]
        xs = st_bits[:, b0:b0 + BL].reshape(NS, BL, CH, P).transpose(3, 2, 0, 1)
        xb[:, :, :SCOL] = xs.reshape(P, CH, SCOL)
        xb[:, :, SCOL] = 0x3F80                          # bf16 1.0
        ts = et_bits[:, b0:b0 + BL].reshape(NT, BL, CH, P).transpose(3, 2, 0, 1)
        in_maps.append({
            "xbuf": xb.reshape(P, CH * XW).view(ml_dtypes.bfloat16),
            "et": np.ascontiguousarray(ts.reshape(P, CH * TCOL)).view(ml_dtypes.float8_e4m3),
        })
    return in_maps


def kernel(student_output, teacher_output, center):
    nc = _get_module()
    in_maps = _prepare_inmaps(student_output, teacher_output, center)
    res = run_bass_kernel_spmd(nc, in_maps, list(range(NCORES))).results

    dots = np.zeros((NT, NS, B))
    z = np.zeros((NT, B))
    lse_sum = np.zeros((NS, B))
    for core in range(NCORES):
        b0 = core * BL
        o = np.asarray(res[core]["out"], dtype=np.float64)  # [128, 161]
        pair = o[0:32].reshape(NT, BL, XW) + o[32:64].reshape(NT, BL, XW)
        dots[:, :, b0:b0 + BL] = np.einsum(
            "tbsb->tsb", pair[:, :, :SCOL].reshape(NT, BL, NS, BL)
        )
        z[:, b0:b0 + BL] = pair[:, :, SCOL]
        lrow = o[64] + o[96]
        lse_sum[:, b0:b0 + BL] = lrow[:SCOL].reshape(NS, BL)

    lse = np.log(lse_sum)
    term = dots / (z[:, None, :] * STUDENT_TEMP)
    M = -(term.mean(axis=-1) - lse.mean(axis=-1)[None, :])
    skip = np.arange(NT)[:, None] == np.arange(NS)[None, :]
    dino = np.where(skip, 0.0, M).sum() / (NT * NS - min(NT, NS))

    e0 = np.asarray(student_output, dtype=np.float32)[0, :NS].astype(np.float64)
    e0 = e0 / np.maximum(np.linalg.norm(e0, axis=-1, keepdims=True), 1e-12)
    sim = e0 @ e0.T
    iu = np.triu(np.ones((NS, NS)), k=1)
    corr = (np.maximum(sim - (1.0 - MARGIN), 0.0) * iu).sum() / (NS * (NS - 1) // 2)

    return np.float32(dino + CORR_WEIGHT * corr)
